# revision 8
# baseline (speedup 1.0000x reference)
"""Trainium2 Bass kernel for nn_Encoder (dense transformer encoder layer).

Sharding: 8 NeuronCores, sequence-parallel. B*S = 2*2048 = 4096 rows ->
512 rows per core; cores 0-3 handle batch 0, cores 4-7 batch 1. Each core
computes Q/K/V for its own rows, AllGathers K^T and V within its 4-core
batch group, then runs attention for its rows over all 16 heads, plus
Wo / LN1 / FFN / LN2 locally (no further communication).

Dataflow is feature-on-partition ("transposed") for all matmuls, f32r
dtype (full PE rate, ~1e-4 relative error):
  x [512,1024] --PE transpose--> xT [1024,512]
  QT/KT per head-pair [128, 512] = Wq_slice.T @ xT
  V natural [512, 1024] = xT_tile.T @ Wv_slice
  AllGather KT -> [4096, 512] and V -> [2048, 1024] per batch group
  S^T tile [128 keys, 512 q] = KT_slice.T @ QT_head (2-head row packing)
  P^T = exp(0.125 * S^T)   (ACT f32-out, then DVE round-copy to f32r)
  O^T [64,512]  += V_slice.T @ P^T   (2-head column packing)
  den^T [1,512] += ones.T @ P^T      (column offsets 0/32)
  oT = O^T * broadcast(1/den) + bv   (PE broadcast of 1/den)
  mhaT = Wo_slice.T @ oT (+bo) -> PE transpose -> +x -> LN1 -> x1n (f32r)
  x1n --PE transpose--> x1nT;  hT = relu(W1.T @ x1nT + b1)
  ffnT = W2.T @ hT + b2 -> PE transpose -> +x1n -> LN2 -> out [512,1024]
"""

import numpy as np

import concourse.bass as bass
import concourse.mybir as mybir
from concourse import bacc
from concourse.tile import TileContext
from concourse.bass_utils import run_bass_kernel_spmd

F32 = mybir.dt.float32
F32R = mybir.dt.float32r
AF = mybir.ActivationFunctionType
OP = mybir.AluOpType
AX = mybir.AxisListType

B, S, D = 2, 2048, 1024
H, DK, DFF = 16, 64, 4096
NCORES = 8
RPC = S * B // NCORES          # 512 rows per core
NP = H // 2                    # 8 head pairs
NKT = S // 128                 # 16 key tiles
GROUPS = [[0, 1, 2, 3], [4, 5, 6, 7]]

_TCNT = [0]


def _mk(pool, shape, dt, tag):
    _TCNT[0] += 1
    return pool.tile(shape, dt, tag=tag, name=f"t{_TCNT[0]}_{tag}")


def build_nc(n_rep=1, use_collective=True):
    nc = bacc.Bacc(num_devices=NCORES)

    xc_in = nc.dram_tensor("xc", [RPC, D], F32R, kind="ExternalInput")
    wq_in = nc.dram_tensor("wq", [8, 128, D], F32R, kind="ExternalInput")
    wk_in = nc.dram_tensor("wk", [8, 128, D], F32R, kind="ExternalInput")
    wv_in = nc.dram_tensor("wv", [8, 128, D], F32R, kind="ExternalInput")
    wo_in = nc.dram_tensor("wo", [8, 128, D], F32R, kind="ExternalInput")
    w1_in = nc.dram_tensor("w1", [8, 8, 128, 512], F32R, kind="ExternalInput")
    w2_in = nc.dram_tensor("w2", [8, 32, 128, 128], F32R, kind="ExternalInput")
    bq_in = nc.dram_tensor("bq", [D, 1], F32, kind="ExternalInput")
    bk_in = nc.dram_tensor("bk", [D, 1], F32, kind="ExternalInput")
    bv_in = nc.dram_tensor("bv", [D, 1], F32, kind="ExternalInput")
    bo_in = nc.dram_tensor("bo", [D, 1], F32, kind="ExternalInput")
    b1_in = nc.dram_tensor("b1", [DFF, 1], F32, kind="ExternalInput")
    b2_in = nc.dram_tensor("b2", [D, 1], F32, kind="ExternalInput")
    g1_in = nc.dram_tensor("g1", [1, D], F32, kind="ExternalInput")
    be1_in = nc.dram_tensor("be1", [1, D], F32, kind="ExternalInput")
    g2_in = nc.dram_tensor("g2", [1, D], F32, kind="ExternalInput")
    be2_in = nc.dram_tensor("be2", [1, D], F32, kind="ExternalInput")
    ident_in = nc.dram_tensor("ident", [128, 128], F32R, kind="ExternalInput")
    ones_in = nc.dram_tensor("ones1", [128, 64], F32R, kind="ExternalInput")
    out_d = nc.dram_tensor("out", [RPC, D], F32, kind="ExternalOutput")

    ag_bufs = []
    for rep in range(n_rep):
        ag_bufs.append((
            nc.dram_tensor(f"kT_ain{rep}", [D, RPC], F32R),
            nc.dram_tensor(f"kT_aout{rep}", [4 * D, RPC], F32R),
            nc.dram_tensor(f"v_ain{rep}", [RPC, D], F32R),
            nc.dram_tensor(f"v_aout{rep}", [4 * RPC, D], F32R),
        ))

    ins = dict(xc=xc_in, wq=wq_in, wk=wk_in, wv=wv_in, wo=wo_in,
               w1=w1_in, w2=w2_in, bq=bq_in, bk=bk_in, bv=bv_in, bo=bo_in,
               b1=b1_in, b2=b2_in, g1=g1_in, be1=be1_in, g2=g2_in,
               be2=be2_in, ident=ident_in, ones1=ones_in, out=out_d)

    with TileContext(nc) as tc:
        for rep in range(n_rep):
            _body(nc, tc, ins, ag_bufs[rep], use_collective)

    nc.finalize()
    return nc


def _body(nc, tc, ins, ag, use_collective):
    kT_ain, kT_aout, v_ain, v_aout = ag

    with (
        tc.tile_pool(name="outer", bufs=1) as po,
        tc.tile_pool(name="psum", bufs=8, space="PSUM") as pp,
    ):
        # ---- constants ----
        ident = _mk(po, [128, 128], F32R, "ident")
        nc.sync.dma_start(out=ident[:], in_=ins["ident"][:])
        ones64 = _mk(po, [128, 64], F32R, "ones")
        nc.sync.dma_start(out=ones64[:], in_=ins["ones1"][:])
        bias = {}
        for nm, n in (("bq", 8), ("bk", 8), ("bv", 8), ("bo", 8),
                      ("b1", 32), ("b2", 8)):
            t = _mk(po, [128, n], F32, "b_" + nm)
            for i in range(n):
                nc.sync.dma_start(out=t[:, i:i + 1],
                                  in_=ins[nm][i * 128:(i + 1) * 128, :])
            bias[nm] = t
        lnw = {}
        for nm in ("g1", "be1", "g2", "be2"):
            t = _mk(po, [128, D], F32, "ln_" + nm)
            nc.sync.dma_start(out=t[:], in_=ins[nm].broadcast_to([128, D]))
            lnw[nm] = t

        # ---- persistent activations ----
        x_nat = [_mk(po, [128, D], F32R, f"x{r}") for r in range(4)]
        for r in range(4):
            nc.sync.dma_start(out=x_nat[r][:],
                              in_=ins["xc"][r * 128:(r + 1) * 128, :])
        qT = [_mk(po, [128, RPC], F32R, f"qT{p}") for p in range(NP)]
        oT = [_mk(po, [128, RPC], F32R, f"oT{p}") for p in range(NP)]
        x1n = [_mk(po, [128, D], F32R, f"x1n{r}") for r in range(4)]

        # ================= phase 1: xT, Q/K/V projections, AllGather ====
        with tc.tile_pool(name="qkv", bufs=1) as pq:
            xT = [_mk(pq, [128, RPC], F32R, f"xT{dc}") for dc in range(8)]
            for dc in range(8):
                for r in range(4):
                    ps = _mk(pp, [128, 512], F32R, "ps")
                    nc.tensor.transpose(ps[:, 0:128],
                                        x_nat[r][:, dc * 128:(dc + 1) * 128],
                                        ident[:])
                    nc.vector.tensor_copy(xT[dc][:, r * 128:(r + 1) * 128],
                                          ps[:, 0:128])

            kT = [_mk(pq, [128, RPC], F32R, f"kT{p}") for p in range(NP)]
            v_nat = [_mk(pq, [128, D], F32R, f"v{r}") for r in range(4)]

            # Q/K: per half (4 head pairs), stream weight tiles [128,512]
            for nm, dst, b in (("wq", qT, "bq"), ("wk", kT, "bk")):
                for pg in range(2):
                    pss = {p: _mk(pp, [128, 512], F32, "ps")
                           for p in range(pg * 4, pg * 4 + 4)}
                    for dc in range(8):
                        wt = pq.tile([128, 512], F32R, tag="wsb", bufs=4,
                                     name=f"w_{nm}{pg}{dc}")
                        nc.sync.dma_start(
                            out=wt[:],
                            in_=ins[nm][dc, :, pg * 512:(pg + 1) * 512])
                        for j, p in enumerate(sorted(pss)):
                            nc.tensor.matmul(pss[p][:],
                                             wt[:, j * 128:(j + 1) * 128],
                                             xT[dc][:],
                                             start=(dc == 0), stop=(dc == 7))
                    for p in pss:
                        nc.vector.tensor_scalar(dst[p][:], pss[p][:],
                                                bias[b][:, p:p + 1], None,
                                                OP.add)

            # V: natural orientation, stream wv [128,512] halves
            for hf in range(2):
                pss = [_mk(pp, [128, 512], F32, "ps") for _ in range(4)]
                for dc in range(8):
                    wt = pq.tile([128, 512], F32R, tag="wsb", bufs=4,
                                 name=f"w_wv{hf}{dc}")
                    nc.sync.dma_start(
                        out=wt[:],
                        in_=ins["wv"][dc, :, hf * 512:(hf + 1) * 512])
                    for r in range(4):
                        nc.tensor.matmul(pss[r][:],
                                         xT[dc][:, r * 128:(r + 1) * 128],
                                         wt[:], start=(dc == 0), stop=(dc == 7))
                for r in range(4):
                    nc.vector.tensor_copy(
                        v_nat[r][:, hf * 512:(hf + 1) * 512], pss[r][:])

            for p in range(NP):
                nc.sync.dma_start(out=kT_ain[p * 128:(p + 1) * 128, :],
                                  in_=kT[p][:])
            for r in range(4):
                nc.sync.dma_start(out=v_ain[r * 128:(r + 1) * 128, :],
                                  in_=v_nat[r][:])

        if use_collective:
            nc.gpsimd.collective_compute("AllGather", OP.bypass,
                                         ins=[kT_ain[:]], outs=[kT_aout[:]],
                                         replica_groups=GROUPS)
            nc.gpsimd.collective_compute("AllGather", OP.bypass,
                                         ins=[v_ain[:]], outs=[v_aout[:]],
                                         replica_groups=GROUPS)
        else:
            for c in range(4):
                nc.sync.dma_start(out=kT_aout[c * D:(c + 1) * D, :],
                                  in_=kT_ain[:])
                nc.sync.dma_start(out=v_aout[c * RPC:(c + 1) * RPC, :],
                                  in_=v_ain[:])

        # bv in per-head layout: col h holds bv[h*64:(h+1)*64] at partitions 0-63
        bvh = _mk(po, [128, H], F32, "bvh")
        for h in range(H):
            nc.sync.dma_start(out=bvh[0:64, h:h + 1],
                              in_=ins["bv"][h * 64:(h + 1) * 64, :])

        # ================= phase 2: attention =================
        # f32r matmuls reject column tiling, so each head's PV output lives
        # at psum partitions 0:65 (65th row = denominator via the ones
        # column appended to V). Head B is assembled into oT[64:128] by a
        # partition-shifting SBUF->SBUF DMA.
        with tc.tile_pool(name="attn", bufs=1) as pa:
            for p in range(NP):
                kts = []
                for c in range(4):
                    t = pa.tile([128, RPC], F32R, tag="kts", bufs=8,
                                name=f"kts{p}_{c}")
                    nc.sync.dma_start(
                        out=t[:],
                        in_=kT_aout[c * D + p * 128:c * D + (p + 1) * 128, :])
                    kts.append(t)
                # vt: per key-tile 130 cols: [V_A(64) | ones] [V_B(64) | ones]
                vt = pa.tile([128, 16 * 130], F32R, tag="vt", bufs=2,
                             name=f"vt{p}")
                vt3 = vt[:].rearrange("p (c j) -> p c j", j=130)
                for hh in range(2):
                    nc.sync.dma_start(
                        out=vt3[:, :, hh * 65:hh * 65 + 64],
                        in_=v_aout[:, p * 128 + hh * 64:p * 128 + (hh + 1) * 64]
                            .rearrange("(c p) j -> p c j", p=128))
                    nc.sync.dma_start(
                        out=vt3[:, :, hh * 65 + 64:hh * 65 + 65],
                        in_=ins["ones1"][:, None, 0:1]
                            .broadcast_to([128, 16, 1]))

                ps_o = [_mk(pp, [128, 512], F32, "ps") for _ in range(2)]
                for kt in range(NKT):
                    c, ksub = divmod(kt, 4)
                    ps_s = [_mk(pp, [128, 512], F32, "ps") for _ in range(2)]
                    for hh in range(2):
                        nc.tensor.matmul(
                            ps_s[hh][:],
                            kts[c][hh * 64:(hh + 1) * 64,
                                   ksub * 128:(ksub + 1) * 128],
                            qT[p][hh * 64:(hh + 1) * 64, :],
                            start=True, stop=True, skip_group_check=True)
                    for hh in range(2):
                        pt_f = pa.tile([128, RPC], F32, tag="ptf", bufs=3,
                                       name=f"ptf{p}_{kt}_{hh}")
                        nc.scalar.activation(pt_f[:], ps_s[hh][:], AF.Exp,
                                             bias=0.0, scale=0.125)
                        pt_r = pa.tile([128, RPC], F32R, tag="ptr", bufs=3,
                                       name=f"ptr{p}_{kt}_{hh}")
                        nc.vector.tensor_copy(pt_r[:], pt_f[:])
                        nc.tensor.matmul(
                            ps_o[hh][0:65, :],
                            vt[:, kt * 130 + hh * 65:kt * 130 + (hh + 1) * 65],
                            pt_r[:], start=(kt == 0), stop=(kt == NKT - 1),
                            skip_group_check=True)
                # normalize per head: o = O[0:64] * bcast(1/den) + bv_head
                for hh in range(2):
                    h = 2 * p + hh
                    den_sb = pa.tile([128, RPC], F32, tag="den", bufs=2,
                                     name=f"den{p}_{hh}")
                    nc.scalar.copy(den_sb[64:65, :], ps_o[hh][64:65, :])
                    rden = pa.tile([128, RPC], F32R, tag="rden", bufs=2,
                                   name=f"rden{p}_{hh}")
                    with nc.allow_low_precision("f32r rounding of 1/den"):
                        nc.vector.reciprocal(rden[64:65, :], den_sb[64:65, :])
                    ps_b = _mk(pp, [128, 512], F32, "ps")
                    nc.tensor.matmul(ps_b[0:64, :], ones64[64:65, :],
                                     rden[64:65, :], start=True, stop=True,
                                     skip_group_check=True)
                    rb = pa.tile([128, RPC], F32, tag="rb", bufs=2,
                                 name=f"rb{p}_{hh}")
                    nc.scalar.copy(rb[0:64, :], ps_b[0:64, :])
                    if hh == 0:
                        tmp = pa.tile([128, RPC], F32, tag="onorm", bufs=2,
                                      name=f"onorm{p}_{hh}")
                        nc.vector.tensor_tensor(tmp[0:64, :], ps_o[hh][0:64, :],
                                                rb[0:64, :], OP.mult)
                        nc.vector.tensor_scalar(oT[p][0:64, :], tmp[0:64, :],
                                                bvh[0:64, h:h + 1], None,
                                                OP.add)
                    else:
                        stage = pa.tile([128, RPC], F32R, tag="stage", bufs=2,
                                        name=f"stage{p}")
                        tmp = pa.tile([128, RPC], F32, tag="onorm", bufs=2,
                                      name=f"onorm{p}_{hh}")
                        nc.vector.tensor_tensor(tmp[0:64, :], ps_o[hh][0:64, :],
                                                rb[0:64, :], OP.mult)
                        nc.vector.tensor_scalar(stage[0:64, :], tmp[0:64, :],
                                                bvh[0:64, h:h + 1], None,
                                                OP.add)
                        nc.sync.dma_start(out=oT[p][64:128, :],
                                          in_=stage[0:64, :])

        # ================= phase 3: Wo, LN1, FFN, LN2, out ==============
        with tc.tile_pool(name="post", bufs=1) as pf:
            x1 = [_mk(pf, [128, D], F32, f"x1_{r}") for r in range(4)]
            for ocg in range(2):
                pss = {oc: _mk(pp, [128, 512], F32, "ps")
                       for oc in range(ocg * 4, ocg * 4 + 4)}
                for dc in range(8):
                    wt = pf.tile([128, 512], F32R, tag="wosb", bufs=2,
                                 name=f"w_wo{ocg}{dc}")
                    nc.sync.dma_start(
                        out=wt[:],
                        in_=ins["wo"][dc, :, ocg * 512:(ocg + 1) * 512])
                    for j, oc in enumerate(sorted(pss)):
                        nc.tensor.matmul(pss[oc][:],
                                         wt[:, j * 128:(j + 1) * 128],
                                         oT[dc][:],
                                         start=(dc == 0), stop=(dc == 7))
                for oc in pss:
                    mt = pf.tile([128, RPC], F32R, tag="mhaT", bufs=2,
                                 name=f"mhaT{oc}")
                    nc.vector.tensor_scalar(mt[:], pss[oc][:],
                                            bias["bo"][:, oc:oc + 1], None,
                                            OP.add)
                    for r in range(4):
                        ps = _mk(pp, [128, 512], F32R, "ps")
                        nc.tensor.transpose(ps[:, 0:128],
                                            mt[:, r * 128:(r + 1) * 128],
                                            ident[:])
                        nc.vector.tensor_tensor(
                            x1[r][:, oc * 128:(oc + 1) * 128],
                            ps[:, 0:128].bitcast(F32),
                            x_nat[r][:, oc * 128:(oc + 1) * 128].bitcast(F32),
                            OP.add)

            for r in range(4):
                _layernorm(nc, pf, x1n[r], x1[r], lnw["g1"], lnw["be1"])
            # x1nT reuses the oT tags (oT dead after the Wo matmuls)
            x1nT = [_mk(po, [128, RPC], F32R, f"oT{dc}") for dc in range(8)]
            for dc in range(8):
                for r in range(4):
                    ps = _mk(pp, [128, 512], F32R, "ps")
                    nc.tensor.transpose(ps[:, 0:128],
                                        x1n[r][:, dc * 128:(dc + 1) * 128],
                                        ident[:])
                    nc.vector.tensor_copy(x1nT[dc][:, r * 128:(r + 1) * 128],
                                          ps[:, 0:128])

            # FFN1: hT[g] holds 4 ffc slabs side by side [128, 4*512]
            hT = [_mk(pf, [128, 4 * RPC], F32R, f"hT{g}") for g in range(8)]
            for g in range(8):
                pss = [_mk(pp, [128, 512], F32, "ps") for _ in range(4)]
                for dc in range(8):
                    w1t = pf.tile([128, 512], F32R, tag="w1t", bufs=2,
                                  name=f"w1t{g}_{dc}")
                    nc.sync.dma_start(out=w1t[:], in_=ins["w1"][g, dc])
                    for j in range(4):
                        nc.tensor.matmul(pss[j][:],
                                         w1t[:, j * 128:(j + 1) * 128],
                                         x1nT[dc][:],
                                         start=(dc == 0), stop=(dc == 7))
                for j in range(4):
                    f = g * 4 + j
                    nc.scalar.activation(hT[g][:, j * RPC:(j + 1) * RPC],
                                         pss[j][:], AF.Relu,
                                         bias=bias["b1"][:, f:f + 1],
                                         scale=1.0)

            # FFN2 + transpose + residual
            x2 = [_mk(pf, [128, D], F32, f"x1_{r}") for r in range(4)]
            for oc in range(8):
                ps2 = _mk(pp, [128, 512], F32, "ps")
                for qrt in range(4):
                    w2t = pf.tile([128, 8 * 128], F32R, tag="w2t", bufs=2,
                                  name=f"w2t{oc}_{qrt}")
                    nc.sync.dma_start(
                        out=w2t[:].rearrange("p (f j) -> p f j", f=8),
                        in_=ins["w2"][oc, qrt * 8:(qrt + 1) * 8]
                            .rearrange("f p j -> p f j"))
                    for fj in range(8):
                        f = qrt * 8 + fj
                        nc.tensor.matmul(
                            ps2[:], w2t[:, fj * 128:(fj + 1) * 128],
                            hT[f // 4][:, (f % 4) * RPC:(f % 4 + 1) * RPC],
                            start=(f == 0), stop=(f == 31))
                ft = pf.tile([128, RPC], F32R, tag="ffnT", bufs=2,
                             name=f"ffnT{oc}")
                nc.vector.tensor_scalar(ft[:], ps2[:], bias["b2"][:, oc:oc + 1],
                                        None, OP.add)
                for r in range(4):
                    ps = _mk(pp, [128, 512], F32R, "ps")
                    nc.tensor.transpose(ps[:, 0:128],
                                        ft[:, r * 128:(r + 1) * 128], ident[:])
                    nc.vector.tensor_tensor(
                        x2[r][:, oc * 128:(oc + 1) * 128],
                        ps[:, 0:128].bitcast(F32),
                        x1n[r][:, oc * 128:(oc + 1) * 128].bitcast(F32),
                        OP.add)

            # LN2 -> out (outt reuses the x tags; x dead after Wo residual)
            outt = [_mk(po, [128, D], F32, f"x{r}") for r in range(4)]
            for r in range(4):
                _layernorm(nc, pf, outt[r], x2[r], lnw["g2"], lnw["be2"])
                nc.sync.dma_start(out=ins["out"][r * 128:(r + 1) * 128, :],
                                  in_=outt[r][:])


def _layernorm(nc, pool, out, x, g, be):
    """LN along the free dim (D). x [128, 1024] f32; out f32 or f32r."""
    mu = pool.tile([128, 1], F32, tag="ln_mu", bufs=2, name=None)
    nc.vector.reduce_sum(mu[:], x[:], axis=AX.X)
    nc.vector.tensor_scalar_mul(mu[:], mu[:], 1.0 / D)
    t = pool.tile([128, D], F32, tag="ln_t", bufs=2, name=None)
    nc.vector.tensor_scalar(t[:], x[:], mu[:], None, OP.subtract)
    sq = pool.tile([128, 1], F32, tag="ln_sq", bufs=2, name=None)
    sq2 = pool.tile([128, D], F32, tag="ln_sq2", bufs=2, name=None)
    nc.scalar.activation(sq2[:], t[:], AF.Square, bias=0.0, scale=1.0,
                         accum_out=sq[:])
    var = pool.tile([128, 1], F32, tag="ln_var", bufs=2, name=None)
    nc.vector.tensor_scalar(var[:], sq[:], 1.0 / D, 1e-5, OP.mult, OP.add)
    std = pool.tile([128, 1], F32, tag="ln_std", bufs=2, name=None)
    nc.scalar.sqrt(std[:], var[:])
    rstd = pool.tile([128, 1], F32, tag="ln_rstd", bufs=2, name=None)
    nc.vector.reciprocal(rstd[:], std[:])
    t2 = pool.tile([128, D], F32, tag="ln_sq2", bufs=2, name=None)
    nc.vector.tensor_scalar_mul(t2[:], t[:], rstd[:])
    t3 = pool.tile([128, D], F32, tag="ln_t", bufs=2, name=None)
    nc.vector.tensor_tensor(t3[:], t2[:], g[:], OP.mult)
    nc.vector.tensor_tensor(out[:], t3[:], be[:], OP.add)


_LN_CNT = [0]
_orig_ln = _layernorm


def _layernorm(nc, pool, out, x, g, be, _orig=_orig_ln):  # noqa: F811
    # wrap to generate unique tile names (pool.tile needs explicit names)
    _LN_CNT[0] += 1
    n = _LN_CNT[0]

    class _P:
        def tile(self, shape, dt, tag, bufs, name):
            _TCNT[0] += 1
            return pool.tile(shape, dt, tag=tag, bufs=bufs,
                             name=f"ln{n}_{tag}_{_TCNT[0]}")

    return _orig(nc, _P(), out, x, g, be)


def prep_inputs(x, Wq, bq, Wk, bk, Wv, bv, Wo, bo, W1, b1, W2, b2,
                g1, be1, g2, be2):
    """Host-side prep: per-core shards + kernel weight layouts (all f32)."""
    f = np.float32
    wq2 = np.ascontiguousarray(
        np.asarray(Wq, f).transpose(1, 0, 2).reshape(D, D).reshape(8, 128, D))
    wk2 = np.ascontiguousarray(
        np.asarray(Wk, f).transpose(1, 0, 2).reshape(D, D).reshape(8, 128, D))
    wv2 = np.ascontiguousarray(
        np.asarray(Wv, f).transpose(1, 0, 2).reshape(D, D).reshape(8, 128, D))
    wo2 = np.ascontiguousarray(np.asarray(Wo, f).reshape(8, 128, D))
    w12 = np.ascontiguousarray(
        np.asarray(W1, f).reshape(8, 128, 8, 512).transpose(2, 0, 1, 3))
    w22 = np.ascontiguousarray(
        np.asarray(W2, f).reshape(32, 128, 8, 128).transpose(2, 0, 1, 3))
    common = {
        "wq": wq2, "wk": wk2, "wv": wv2, "wo": wo2, "w1": w12, "w2": w22,
        "bq": np.asarray(bq, f).reshape(D, 1),
        "bk": np.asarray(bk, f).reshape(D, 1),
        "bv": np.asarray(bv, f).reshape(D, 1),
        "bo": np.asarray(bo, f).reshape(D, 1),
        "b1": np.asarray(b1, f).reshape(DFF, 1),
        "b2": np.asarray(b2, f).reshape(D, 1),
        "g1": np.asarray(g1, f).reshape(1, D),
        "be1": np.asarray(be1, f).reshape(1, D),
        "g2": np.asarray(g2, f).reshape(1, D),
        "be2": np.asarray(be2, f).reshape(1, D),
        "ident": np.eye(128, dtype=f),
        "ones1": np.ones((128, 64), dtype=f),
    }
    xf = np.asarray(x, f)
    in_maps = []
    for c in range(NCORES):
        b, j = divmod(c, 4)
        m = dict(common)
        m["xc"] = np.ascontiguousarray(xf[b, j * RPC:(j + 1) * RPC, :])
        in_maps.append(m)
    return in_maps


_NC_CACHE = {}


def kernel(**inputs) -> np.ndarray:
    if "main" not in _NC_CACHE:
        _NC_CACHE["main"] = build_nc(n_rep=1, use_collective=True)
    nc = _NC_CACHE["main"]
    in_maps = prep_inputs(**inputs)
    res = run_bass_kernel_spmd(nc, in_maps, core_ids=list(range(NCORES)))
    out = np.empty((B, S, D), np.float32)
    for c in range(NCORES):
        b, j = divmod(c, 4)
        out[b, j * RPC:(j + 1) * RPC, :] = res.results[c]["out"]
    return out
